# revision 8
# baseline (speedup 1.0000x reference)
"""HGNN forward kernel on 8 TRN2 NeuronCores (Bass/Tile, SPMD).

Model (V=8192 nodes, E=4096 hyperedges, DIN=512, DH=256, 2 classes):
    H_tilde = Dv^-1/2 H De^-1/2 ; HHt = H_tilde @ H_tilde.T
    Xh = X @ Wp.T
    2x: Xh = LN(Xh + relu((HHt @ Xh) @ W.T))
    out = softmax((H.T @ Xh / cnt) @ Wc.T + bc)

Key algebra: HHt @ Xh = Dv^-1/2 H De^-1 H.T Dv^-1/2 Xh — never materialize
the (V,V) HHt. Shard V across 8 cores; keep the local H shard (bf16) and its
transpose resident in SBUF; the cross-core contraction over V (P = H.T @ Xhs)
is a 2MB bf16 AllReduce per layer.
"""

import numpy as np

import concourse.bass as bass
import concourse.mybir as mybir
import concourse.tile as tile
from concourse import bacc
from concourse.bass_utils import run_bass_kernel_spmd

V, E, DIN, DH, NCLS = 8192, 4096, 512, 256, 2
NCORES = 8
VL = V // NCORES           # 1024 local nodes
VK = VL // 128             # 8 local v-tiles
EK = E // 128              # 32 e-tiles
DK = DIN // 128            # 4
HK = DH // 128             # 2
EPS = 1e-5
F32 = mybir.dt.float32
BF16 = mybir.dt.bfloat16
AX = mybir.AxisListType.X
AF = mybir.ActivationFunctionType
OP = mybir.AluOpType
RG = [list(range(NCORES))]


def _bcast_ap(dram_ap, parts, extra=None):
    """Partition-broadcast read AP for a DRAM vector."""
    ap = [[0, parts]]
    if extra is not None:
        ap.append([0, extra])
    ap.extend(dram_ap.ap)
    return bass.AP(tensor=dram_ap.tensor, offset=dram_ap.offset, ap=ap)


def _body(tc):
    from contextlib import ExitStack
    ctx = ExitStack()
    nc = tc.nc
    X_d = nc.dram_tensor("X", [VL, DIN], F32, kind="ExternalInput")
    H_d = nc.dram_tensor("H", [VL, E], F32, kind="ExternalInput")
    Wp_d = nc.dram_tensor("Wp", [DH, DIN], F32, kind="ExternalInput")
    W0_d = nc.dram_tensor("W0", [DH, DH], F32, kind="ExternalInput")
    W1_d = nc.dram_tensor("W1", [DH, DH], F32, kind="ExternalInput")
    g0_d = nc.dram_tensor("g0", [DH], F32, kind="ExternalInput")
    b0_d = nc.dram_tensor("b0", [DH], F32, kind="ExternalInput")
    g1_d = nc.dram_tensor("g1", [DH], F32, kind="ExternalInput")
    b1_d = nc.dram_tensor("b1", [DH], F32, kind="ExternalInput")
    Wc_d = nc.dram_tensor("Wc", [NCLS, DH], F32, kind="ExternalInput")
    bc_d = nc.dram_tensor("bc", [NCLS], F32, kind="ExternalInput")
    out_d = nc.dram_tensor("out", [E, NCLS], F32, kind="ExternalOutput")

    P = 128
    persist = ctx.enter_context(tc.tile_pool(name="persist", bufs=1))
    stage = ctx.enter_context(tc.tile_pool(name="stage", bufs=2))
    qstage = ctx.enter_context(tc.tile_pool(name="qstage", bufs=3))
    epi = ctx.enter_context(tc.tile_pool(name="epi", bufs=3))
    psum_mm = ctx.enter_context(tc.tile_pool(name="psum_mm", bufs=6, space="PSUM"))
    psum_sm = ctx.enter_context(tc.tile_pool(name="psum_sm", bufs=2, space="PSUM"))
    dram = ctx.enter_context(tc.tile_pool(name="dram", bufs=1, space="DRAM"))

    # ---- persistent SBUF state ----
    H_sb = persist.tile([P, VK, E], BF16)      # H rows (local shard)
    HT_sb = persist.tile([P, EK, VL], BF16)    # H.T (local columns)
    XT_sb = persist.tile([P, DK, VL], BF16)    # X.T
    Xh = persist.tile([P, VK, DH], F32)        # residual stream (f32)
    Xhs = persist.tile([P, VK, DH], BF16)      # matmul operand (scaled/cast)
    Ps = persist.tile([P, EK, DH], BF16)       # P = H.T @ Xhs (post-AR)
    QT = persist.tile([P, HK, VL], BF16)       # Q.T (also reused for Xh_f.T)
    WpT = persist.tile([P, DK, DH], BF16)
    W0T = persist.tile([P, HK, DH], BF16)
    W1T = persist.tile([P, HK, DH], BF16)
    WcT = persist.tile([P, HK, 16], BF16)      # only [:, :, 0:2] valid
    ybf = persist.tile([P, VK, NCLS], BF16)
    g0b = persist.tile([P, DH], F32)
    b0b = persist.tile([P, DH], F32)
    g1b = persist.tile([P, DH], F32)
    b1b = persist.tile([P, DH], F32)
    bc2 = persist.tile([P, EK, NCLS], F32)
    dv = persist.tile([P, VK], F32)
    dvi = persist.tile([P, VK], F32)
    de_p = persist.tile([P, EK], F32)          # local partial
    de_g = persist.tile([P, EK], F32)          # global (post-AR)
    dei2 = persist.tile([P, EK], F32)
    cnt2 = persist.tile([P, EK, NCLS], F32)
    pooled = persist.tile([P, EK, NCLS], F32)
    outsb = persist.tile([P, EK, NCLS], F32)
    eps_t = persist.tile([P, 1], F32)

    # ---- DRAM bounce buffers for collectives ----
    de_in = dram.tile([P, EK], F32)
    de_out = dram.tile([P, EK], F32, addr_space="Shared")
    P_in = [dram.tile([P, EK, DH], BF16, name=f"P_in{i}") for i in range(2)]
    P_out = [dram.tile([P, EK, DH], BF16, addr_space="Shared", name=f"P_out{i}")
             for i in range(2)]
    pl_in = dram.tile([P, EK, NCLS], F32)
    pl_out = dram.tile([P, EK, NCLS], F32, addr_space="Shared")

    nc.vector.memset(eps_t, EPS)

    # ---- small-input loads (start early; cheap) ----
    nc.gpsimd.dma_start(out=g0b, in_=_bcast_ap(g0_d[:], P))
    nc.gpsimd.dma_start(out=b0b, in_=_bcast_ap(b0_d[:], P))
    nc.gpsimd.dma_start(out=g1b, in_=_bcast_ap(g1_d[:], P))
    nc.gpsimd.dma_start(out=b1b, in_=_bcast_ap(b1_d[:], P))
    nc.gpsimd.dma_start(out=bc2, in_=_bcast_ap(bc_d[:], P, extra=EK))

    wp_st = stage.tile([P, HK, DIN], BF16, tag="wp_st")
    nc.gpsimd.dma_start(out=wp_st, in_=Wp_d.rearrange("(t p) f -> p t f", p=P))
    for t in range(HK):
        for dk in range(DK):
            nc.sync.dma_start(
                out=WpT[:, dk, t * P:(t + 1) * P],
                in_=wp_st[:, t, dk * P:(dk + 1) * P],
                transpose=True,
            )
    for W_dram, WT in ((W0_d, W0T), (W1_d, W1T)):
        w_st = stage.tile([P, HK, DH], BF16, tag="w_st")
        nc.gpsimd.dma_start(out=w_st, in_=W_dram.rearrange("(t p) f -> p t f", p=P))
        for t in range(HK):
            for kt in range(HK):
                nc.sync.dma_start(
                    out=WT[:, kt, t * P:(t + 1) * P],
                    in_=w_st[:, t, kt * P:(kt + 1) * P],
                    transpose=True,
                )
    wc_st = stage.tile([16, DH], BF16, tag="wc_st")
    nc.vector.memset(wc_st, 0.0)
    nc.gpsimd.dma_start(out=wc_st[0:NCLS, :], in_=Wc_d[:, :])
    for kt in range(HK):
        nc.sync.dma_start(
            out=WcT[:, kt, :],
            in_=wc_st[:, kt * P:(kt + 1) * P],
            transpose=True,
        )

    # X: cast to bf16 and transpose
    for vk in range(VK):
        xb = stage.tile([P, DIN], BF16, tag="xb")
        nc.gpsimd.dma_start(out=xb, in_=X_d[vk * P:(vk + 1) * P, :])
        for dk in range(DK):
            nc.sync.dma_start(
                out=XT_sb[:, dk, vk * P:(vk + 1) * P],
                in_=xb[:, dk * P:(dk + 1) * P],
                transpose=True,
            )

    # ---- phase 1: stream H (cast f32->bf16 in DMA), degrees, H.T ----
    for vk in range(VK):
        nc.gpsimd.dma_start(out=H_sb[:, vk, :], in_=H_d[vk * P:(vk + 1) * P, :])
        nc.vector.reduce_sum(dv[:, vk:vk + 1], H_sb[:, vk, :], axis=AX)
    for vk in range(VK):
        for et in range(EK):
            nc.sync.dma_start(
                out=HT_sb[:, et, vk * P:(vk + 1) * P],
                in_=H_sb[:, vk, et * P:(et + 1) * P],
                transpose=True,
            )
    for et in range(EK):
        nc.vector.reduce_sum(de_p[:, et:et + 1], HT_sb[:, et, :], axis=AX)

    # d_e AllReduce (16 KB)
    nc.gpsimd.dma_start(out=de_in[:], in_=de_p[:])
    nc.gpsimd.collective_compute(
        "AllReduce", OP.add, replica_groups=RG, ins=[de_in.opt()], outs=[de_out.opt()]
    )
    nc.gpsimd.dma_start(out=de_g[:], in_=de_out[:])

    # dei2 = d_e / max(d_e,1)^2  (== 1/d_e, 0 where empty)
    m_e = epi.tile([P, EK], F32, tag="m_e")
    nc.vector.tensor_scalar_max(m_e, de_g[:], 1.0)
    sq_e = epi.tile([P, EK], F32, tag="sq_e")
    nc.vector.tensor_mul(sq_e, m_e, m_e)
    nc.vector.reciprocal(sq_e, sq_e)
    nc.vector.tensor_mul(dei2[:], de_g[:], sq_e)
    # cnt2 = 1/max(d_e,1), interleaved per class pair
    nc.vector.reciprocal(m_e, m_e)
    for c in range(NCLS):
        nc.vector.tensor_copy(cnt2[:, :, c], m_e)

    # dvi = d_v / max(d_v,1)^1.5  (== d_v^-0.5, 0 where isolated)
    m_v = epi.tile([P, VK], F32, tag="m_v")
    nc.vector.tensor_scalar_max(m_v, dv[:], 1.0)
    s_v = epi.tile([P, VK], F32, tag="s_v")
    nc.scalar.sqrt(s_v, m_v)
    nc.vector.tensor_mul(m_v, m_v, s_v)
    nc.vector.reciprocal(m_v, m_v)
    nc.vector.tensor_mul(dvi[:], dv[:], m_v)

    # ---- Xh init: Xh = X @ Wp.T ; Xhs = dvi * Xh (bf16) ----
    for m in range(VK):
        ps = psum_mm.tile([P, DH], F32, tag="ps")
        for dk in range(DK):
            nc.tensor.matmul(
                ps, XT_sb[:, dk, m * P:(m + 1) * P], WpT[:, dk, :],
                start=(dk == 0), stop=(dk == DK - 1),
            )
        nc.scalar.copy(Xh[:, m, :], ps)
        nc.vector.tensor_scalar_mul(Xhs[:, m, :], Xh[:, m, :], dvi[:, m:m + 1])

    # ---- two message-passing layers ----
    for layer, (WT, gb, bb) in enumerate(((W0T, g0b, b0b), (W1T, g1b, b1b))):
        # P = H.T @ Xhs   (contraction over local V; AllReduce over cores)
        for m in range(EK):
            ps = psum_mm.tile([P, DH], F32, tag="ps")
            for k in range(VK):
                nc.tensor.matmul(
                    ps, H_sb[:, k, m * P:(m + 1) * P], Xhs[:, k, :],
                    start=(k == 0), stop=(k == VK - 1),
                )
            nc.scalar.copy(Ps[:, m, :], ps)
        nc.gpsimd.dma_start(out=P_in[layer][:], in_=Ps[:])
        nc.gpsimd.collective_compute(
            "AllReduce", OP.add, replica_groups=RG,
            ins=[P_in[layer].opt()], outs=[P_out[layer].opt()],
        )
        nc.gpsimd.dma_start(out=Ps[:], in_=P_out[layer][:])
        # scale rows by 1/d_e
        for et in range(EK):
            nc.vector.tensor_scalar_mul(Ps[:, et, :], Ps[:, et, :], dei2[:, et:et + 1])
        # Q = H @ Ps  -> Q.T via DMA transpose
        for m in range(VK):
            ps = psum_mm.tile([P, DH], F32, tag="ps")
            for k in range(EK):
                nc.tensor.matmul(
                    ps, HT_sb[:, k, m * P:(m + 1) * P], Ps[:, k, :],
                    start=(k == 0), stop=(k == EK - 1),
                )
            qm = qstage.tile([P, DH], BF16, tag="qm")
            nc.scalar.copy(qm, ps)
            for kt in range(HK):
                nc.sync.dma_start(
                    out=QT[:, kt, m * P:(m + 1) * P],
                    in_=qm[:, kt * P:(kt + 1) * P],
                    transpose=True,
                )
        # R = relu(dvi*(Q @ W.T)) ; Xh = LN(Xh + R) ; refresh Xhs
        for m in range(VK):
            ps = psum_mm.tile([P, DH], F32, tag="ps")
            for kt in range(HK):
                nc.tensor.matmul(
                    ps, QT[:, kt, m * P:(m + 1) * P], WT[:, kt, :],
                    start=(kt == 0), stop=(kt == HK - 1),
                )
            s = epi.tile([P, DH], F32, tag="s")
            nc.scalar.activation(s, ps, AF.Relu, scale=dvi[:, m:m + 1])
            nc.vector.tensor_add(s, s, Xh[:, m, :])
            stats = epi.tile([P, 6], F32, tag="stats")
            nc.vector.bn_stats(stats, s)
            mv = epi.tile([P, 2], F32, tag="mv")
            nc.vector.bn_aggr(mv, stats)
            sd = epi.tile([P, 1], F32, tag="sd")
            nc.scalar.activation(sd, mv[:, 1:2], AF.Sqrt, bias=eps_t[:])
            nc.vector.reciprocal(sd, sd)
            nc.vector.tensor_scalar(
                s, s, scalar1=mv[:, 0:1], scalar2=sd,
                op0=OP.subtract, op1=OP.mult,
            )
            nc.vector.tensor_mul(s, s, gb)
            nc.vector.tensor_add(Xh[:, m, :], s, bb)
            if layer == 0:
                nc.vector.tensor_scalar_mul(Xhs[:, m, :], Xh[:, m, :], dvi[:, m:m + 1])
            else:
                nc.vector.tensor_copy(Xhs[:, m, :], Xh[:, m, :])

    # ---- pooling + classifier + softmax ----
    # Xh_f.T (into QT), y = Xh_f @ Wc.T
    for m in range(VK):
        for kt in range(HK):
            nc.sync.dma_start(
                out=QT[:, kt, m * P:(m + 1) * P],
                in_=Xhs[:, m, kt * P:(kt + 1) * P],
                transpose=True,
            )
    for m in range(VK):
        ps = psum_sm.tile([P, NCLS], F32, tag="psy")
        for kt in range(HK):
            nc.tensor.matmul(
                ps, QT[:, kt, m * P:(m + 1) * P], WcT[:, kt, 0:NCLS],
                start=(kt == 0), stop=(kt == HK - 1),
            )
        nc.scalar.copy(ybf[:, m, :], ps)
    # pooled = H.T @ y (partial over local V)
    for m in range(EK):
        ps = psum_sm.tile([P, NCLS], F32, tag="psy")
        for k in range(VK):
            nc.tensor.matmul(
                ps, H_sb[:, k, m * P:(m + 1) * P], ybf[:, k, :],
                start=(k == 0), stop=(k == VK - 1),
            )
        nc.scalar.copy(pooled[:, m, :], ps)
    nc.gpsimd.dma_start(out=pl_in[:], in_=pooled[:])
    nc.gpsimd.collective_compute(
        "AllReduce", OP.add, replica_groups=RG, ins=[pl_in.opt()], outs=[pl_out.opt()]
    )
    nc.gpsimd.dma_start(out=pooled[:], in_=pl_out[:])
    # logits = pooled/cnt + bc ; softmax over the class pair
    nc.vector.tensor_mul(pooled[:], pooled[:], cnt2[:])
    nc.vector.tensor_add(pooled[:], pooled[:], bc2[:])
    nc.scalar.activation(outsb[:], pooled[:], AF.Exp)
    ssum = epi.tile([P, EK], F32, tag="ssum")
    nc.vector.tensor_add(ssum, outsb[:, :, 0], outsb[:, :, 1])
    nc.vector.reciprocal(ssum, ssum)
    for c in range(NCLS):
        nc.vector.tensor_mul(outsb[:, :, c], outsb[:, :, c], ssum)
    nc.sync.dma_start(
        out=out_d.rearrange("(et p) c -> p et c", p=P),
        in_=outsb[:],
    )
    ctx.close()


_NC_CACHE = None


def _build():
    global _NC_CACHE
    if _NC_CACHE is not None:
        return _NC_CACHE
    nc = bacc.Bacc("TRN2", target_bir_lowering=False, debug=False, num_devices=NCORES)
    with tile.TileContext(nc) as tc:
        _body(tc)
    nc.compile()
    _NC_CACHE = nc
    return nc


def kernel(**inputs):
    X = np.ascontiguousarray(np.asarray(inputs["X"], dtype=np.float32))
    H = np.ascontiguousarray(np.asarray(inputs["H_inc"], dtype=np.float32))
    full = {
        "Wp": inputs["Wp"], "W0": inputs["W0"], "W1": inputs["W1"],
        "g0": inputs["g0"], "b0": inputs["b0"],
        "g1": inputs["g1"], "b1": inputs["b1"],
        "Wc": inputs["Wc"], "bc": inputs["bc"],
    }
    full = {k: np.ascontiguousarray(np.asarray(v, dtype=np.float32))
            for k, v in full.items()}
    nc = _build()
    in_maps = []
    for c in range(NCORES):
        sl = slice(c * VL, (c + 1) * VL)
        m = {"X": X[sl], "H": H[sl]}
        m.update(full)
        in_maps.append(m)
    res = run_bass_kernel_spmd(
        nc, in_maps, core_ids=list(range(NCORES)), **_RUN_KWARGS
    )
    if _LAST_RESULT is not None:
        _LAST_RESULT.clear()
        _LAST_RESULT.append(res)
    return np.asarray(res.results[0]["out"], dtype=np.float32)


# test-harness hooks (unused during grading)
_RUN_KWARGS = {}
_LAST_RESULT = None


# revision 9
# speedup vs baseline: 1.5447x; 1.5447x over previous
"""HGNN forward kernel on 8 TRN2 NeuronCores (Bass/Tile, SPMD).

Model (V=8192 nodes, E=4096 hyperedges, DIN=512, DH=256, 2 classes):
    H_tilde = Dv^-1/2 H De^-1/2 ; HHt = H_tilde @ H_tilde.T
    Xh = X @ Wp.T
    2x: Xh = LN(Xh + relu((HHt @ Xh) @ W.T))
    out = softmax((H.T @ Xh / cnt) @ Wc.T + bc)

Key algebra: HHt @ Xh = Dv^-1/2 H De^-1 H.T Dv^-1/2 Xh — never materialize
the (V,V) HHt. Shard V across 8 cores; keep the local H shard (bf16) and its
transpose resident in SBUF; the cross-core contraction over V (P = H.T @ Xhs)
is a 2MB bf16 AllReduce per layer, chunked in 2 to overlap with the Q matmul.
"""

import numpy as np

import concourse.bass as bass
import concourse.mybir as mybir
import concourse.tile as tile
from concourse import bacc
from concourse.bass_utils import run_bass_kernel_spmd

V, E, DIN, DH, NCLS = 8192, 4096, 512, 256, 2
NCORES = 8
VL = V // NCORES           # 1024 local nodes
VK = VL // 128             # 8 local v-tiles
EK = E // 128              # 32 e-tiles
DK = DIN // 128            # 4
HK = DH // 128             # 2
ECH = 2                    # E-chunks for AllReduce overlap
EKC = EK // ECH            # e-tiles per chunk
EPS = 1e-5
F32 = mybir.dt.float32
BF16 = mybir.dt.bfloat16
AX = mybir.AxisListType.X
AF = mybir.ActivationFunctionType
OP = mybir.AluOpType
RG = [list(range(NCORES))]


def _bcast_ap(dram_ap, parts, extra=None):
    """Partition-broadcast read AP for a DRAM vector."""
    ap = [[0, parts]]
    if extra is not None:
        ap.append([0, extra])
    ap.extend(dram_ap.ap)
    return bass.AP(tensor=dram_ap.tensor, offset=dram_ap.offset, ap=ap)


def _body(tc):
    from contextlib import ExitStack
    ctx = ExitStack()
    nc = tc.nc
    X_d = nc.dram_tensor("X", [VL, DIN], F32, kind="ExternalInput")
    H_d = nc.dram_tensor("H", [VL, E], F32, kind="ExternalInput")
    Wp_d = nc.dram_tensor("Wp", [DH, DIN], F32, kind="ExternalInput")
    W0_d = nc.dram_tensor("W0", [DH, DH], F32, kind="ExternalInput")
    W1_d = nc.dram_tensor("W1", [DH, DH], F32, kind="ExternalInput")
    g0_d = nc.dram_tensor("g0", [DH], F32, kind="ExternalInput")
    b0_d = nc.dram_tensor("b0", [DH], F32, kind="ExternalInput")
    g1_d = nc.dram_tensor("g1", [DH], F32, kind="ExternalInput")
    b1_d = nc.dram_tensor("b1", [DH], F32, kind="ExternalInput")
    Wc_d = nc.dram_tensor("Wc", [NCLS, DH], F32, kind="ExternalInput")
    bc_d = nc.dram_tensor("bc", [NCLS], F32, kind="ExternalInput")
    out_d = nc.dram_tensor("out", [E, NCLS], F32, kind="ExternalOutput")

    P = 128
    persist = ctx.enter_context(tc.tile_pool(name="persist", bufs=1))
    stage = ctx.enter_context(tc.tile_pool(name="stage", bufs=2))
    qstage = ctx.enter_context(tc.tile_pool(name="qstage", bufs=3))
    epi = ctx.enter_context(tc.tile_pool(name="epi", bufs=3))
    psum_mm = ctx.enter_context(tc.tile_pool(name="psum_mm", bufs=6, space="PSUM"))
    psum_sm = ctx.enter_context(tc.tile_pool(name="psum_sm", bufs=2, space="PSUM"))
    dram = ctx.enter_context(tc.tile_pool(name="dram", bufs=1, space="DRAM"))

    # ---- persistent SBUF state ----
    H_sb = persist.tile([P, VK, E], BF16)      # H rows (local shard)
    HT_sb = persist.tile([P, EK, VL], BF16)    # H.T (local columns)
    XT_sb = persist.tile([P, DK, VL], BF16)    # X.T
    Xh = persist.tile([P, VK, DH], F32)        # residual stream (f32)
    Xhs = persist.tile([P, VK, DH], BF16)      # matmul operand (scaled/cast)
    Ps = persist.tile([P, EK, DH], BF16)       # P = H.T @ Xhs (post-AR)
    QT = persist.tile([P, HK, VL], BF16)       # Q.T (also reused for Xh_f.T)
    WpT = persist.tile([P, DK, DH], BF16)
    W0T = persist.tile([P, HK, DH], BF16)
    W1T = persist.tile([P, HK, DH], BF16)
    WcT = persist.tile([P, HK, 16], BF16)      # only [:, :, 0:2] valid
    ybf = persist.tile([P, VK, NCLS], BF16)
    g0b = persist.tile([P, DH], F32)
    b0b = persist.tile([P, DH], F32)
    g1b = persist.tile([P, DH], F32)
    b1b = persist.tile([P, DH], F32)
    bc2 = persist.tile([P, EK, NCLS], F32)
    dv = persist.tile([P, VK], F32)
    dvi = persist.tile([P, VK], F32)
    de_p = persist.tile([P, EK], F32)          # local partial
    de_g = persist.tile([P, EK], F32)          # global (post-AR)
    dei2 = persist.tile([P, EK], F32)
    cnt2 = persist.tile([P, EK, NCLS], F32)
    pooled = persist.tile([P, EK, NCLS], F32)
    outsb = persist.tile([P, EK, NCLS], F32)
    eps_t = persist.tile([P, 1], F32)

    # ---- DRAM bounce buffers for collectives ----
    de_in = dram.tile([P, EK], F32)
    de_out = dram.tile([P, EK], F32, addr_space="Shared")
    P_in = [[dram.tile([P, EKC, DH], BF16, name=f"P_in{l}_{c}")
             for c in range(ECH)] for l in range(2)]
    P_out = [[dram.tile([P, EKC, DH], BF16, addr_space="Shared",
                        name=f"P_out{l}_{c}")
              for c in range(ECH)] for l in range(2)]
    pl_in = dram.tile([P, EK, NCLS], F32)
    pl_out = dram.tile([P, EK, NCLS], F32, addr_space="Shared")

    nc.vector.memset(eps_t, EPS)

    # ---- small-input loads (start early; cheap) ----
    nc.gpsimd.dma_start(out=g0b, in_=_bcast_ap(g0_d[:], P))
    nc.gpsimd.dma_start(out=b0b, in_=_bcast_ap(b0_d[:], P))
    nc.gpsimd.dma_start(out=g1b, in_=_bcast_ap(g1_d[:], P))
    nc.gpsimd.dma_start(out=b1b, in_=_bcast_ap(b1_d[:], P))
    nc.gpsimd.dma_start(out=bc2, in_=_bcast_ap(bc_d[:], P, extra=EK))

    wp_st = stage.tile([P, HK, DIN], BF16, tag="wp_st")
    nc.gpsimd.dma_start(out=wp_st, in_=Wp_d.rearrange("(t p) f -> p t f", p=P))
    for t in range(HK):
        nc.sync.dma_start(
            out=WpT[:, :, t * P:(t + 1) * P], in_=wp_st[:, t, :], transpose=True
        )
    for W_dram, WT in ((W0_d, W0T), (W1_d, W1T)):
        w_st = stage.tile([P, HK, DH], BF16, tag="w_st")
        nc.gpsimd.dma_start(out=w_st, in_=W_dram.rearrange("(t p) f -> p t f", p=P))
        for t in range(HK):
            nc.sync.dma_start(
                out=WT[:, :, t * P:(t + 1) * P], in_=w_st[:, t, :], transpose=True
            )
    wc_st = stage.tile([16, DH], BF16, tag="wc_st")
    nc.vector.memset(wc_st, 0.0)
    nc.gpsimd.dma_start(out=wc_st[0:NCLS, :], in_=Wc_d[:, :])
    nc.sync.dma_start(out=WcT[:, :, :], in_=wc_st[:, :], transpose=True)

    # X: cast to bf16 and transpose (one DMA-transpose per v-tile)
    for vk in range(VK):
        xb = stage.tile([P, DIN], BF16, tag="xb")
        nc.gpsimd.dma_start(out=xb, in_=X_d[vk * P:(vk + 1) * P, :])
        nc.sync.dma_start(
            out=XT_sb[:, :, vk * P:(vk + 1) * P], in_=xb[:, :], transpose=True
        )

    # ---- phase 1: stream H (cast f32->bf16 in DMA), degrees, H.T ----
    for vk in range(VK):
        nc.gpsimd.dma_start(out=H_sb[:, vk, :], in_=H_d[vk * P:(vk + 1) * P, :])
        nc.vector.reduce_sum(dv[:, vk:vk + 1], H_sb[:, vk, :], axis=AX)
        nc.sync.dma_start(
            out=HT_sb[:, :, vk * P:(vk + 1) * P], in_=H_sb[:, vk, :], transpose=True
        )
    nc.vector.reduce_sum(de_p[:, :], HT_sb[:, :, :], axis=AX)

    # d_e AllReduce (16 KB)
    nc.sync.dma_start(out=de_in[:], in_=de_p[:])
    nc.gpsimd.collective_compute(
        "AllReduce", OP.add, replica_groups=RG, ins=[de_in.opt()], outs=[de_out.opt()]
    )
    nc.sync.dma_start(out=de_g[:], in_=de_out[:])

    # dei2 = d_e / max(d_e,1)^2  (== 1/d_e, 0 where empty)
    m_e = epi.tile([P, EK], F32, tag="m_e")
    nc.vector.tensor_scalar_max(m_e, de_g[:], 1.0)
    sq_e = epi.tile([P, EK], F32, tag="sq_e")
    nc.vector.tensor_mul(sq_e, m_e, m_e)
    nc.vector.reciprocal(sq_e, sq_e)
    nc.vector.tensor_mul(dei2[:], de_g[:], sq_e)
    # cnt2 = 1/max(d_e,1), interleaved per class pair
    nc.vector.reciprocal(m_e, m_e)
    for c in range(NCLS):
        nc.vector.tensor_copy(cnt2[:, :, c], m_e)

    # dvi = d_v / max(d_v,1)^1.5  (== d_v^-0.5, 0 where isolated)
    m_v = epi.tile([P, VK], F32, tag="m_v")
    nc.vector.tensor_scalar_max(m_v, dv[:], 1.0)
    s_v = epi.tile([P, VK], F32, tag="s_v")
    nc.scalar.sqrt(s_v, m_v)
    nc.vector.tensor_mul(m_v, m_v, s_v)
    nc.vector.reciprocal(m_v, m_v)
    nc.vector.tensor_mul(dvi[:], dv[:], m_v)

    # ---- Xh init: Xh = X @ Wp.T ; Xhs = dvi * Xh (bf16) ----
    for m in range(VK):
        ps = psum_mm.tile([P, DH], F32, tag="ps")
        for dk in range(DK):
            nc.tensor.matmul(
                ps, XT_sb[:, dk, m * P:(m + 1) * P], WpT[:, dk, :],
                start=(dk == 0), stop=(dk == DK - 1),
            )
        nc.scalar.copy(Xh[:, m, :], ps)
        nc.vector.tensor_scalar_mul(Xhs[:, m, :], Xh[:, m, :], dvi[:, m:m + 1])

    # ---- two message-passing layers ----
    for layer, (WT, gb, bb) in enumerate(((W0T, g0b, b0b), (W1T, g1b, b1b))):
        # P = H.T @ Xhs  (contraction over local V; AllReduce over cores),
        # chunked over E so the AllReduce overlaps the next chunk's matmuls
        # and the Q matmuls below.
        for c in range(ECH):
            for m in range(c * EKC, (c + 1) * EKC):
                ps = psum_mm.tile([P, DH], F32, tag="ps")
                for k in range(VK):
                    nc.tensor.matmul(
                        ps, H_sb[:, k, m * P:(m + 1) * P], Xhs[:, k, :],
                        start=(k == 0), stop=(k == VK - 1),
                    )
                nc.scalar.copy(Ps[:, m, :], ps)
            nc.sync.dma_start(
                out=P_in[layer][c][:], in_=Ps[:, c * EKC:(c + 1) * EKC, :]
            )
            nc.gpsimd.collective_compute(
                "AllReduce", OP.add, replica_groups=RG,
                ins=[P_in[layer][c].opt()], outs=[P_out[layer][c].opt()],
            )
        for c in range(ECH):
            nc.sync.dma_start(
                out=Ps[:, c * EKC:(c + 1) * EKC, :], in_=P_out[layer][c][:]
            )
            for et in range(c * EKC, (c + 1) * EKC):
                nc.vector.tensor_scalar_mul(
                    Ps[:, et, :], Ps[:, et, :], dei2[:, et:et + 1]
                )
        # Q = H @ Ps  -> Q.T via DMA transpose
        for m in range(VK):
            ps = psum_mm.tile([P, DH], F32, tag="ps")
            for k in range(EK):
                nc.tensor.matmul(
                    ps, HT_sb[:, k, m * P:(m + 1) * P], Ps[:, k, :],
                    start=(k == 0), stop=(k == EK - 1),
                )
            qm = qstage.tile([P, DH], BF16, tag="qm")
            nc.scalar.copy(qm, ps)
            nc.sync.dma_start(
                out=QT[:, :, m * P:(m + 1) * P], in_=qm[:, :], transpose=True
            )
        # R = relu(dvi*(Q @ W.T)) ; Xh = LN(Xh + R) ; refresh Xhs
        for m in range(VK):
            ps = psum_mm.tile([P, DH], F32, tag="ps")
            for kt in range(HK):
                nc.tensor.matmul(
                    ps, QT[:, kt, m * P:(m + 1) * P], WT[:, kt, :],
                    start=(kt == 0), stop=(kt == HK - 1),
                )
            s = epi.tile([P, DH], F32, tag="s")
            nc.scalar.activation(s, ps, AF.Relu, scale=dvi[:, m:m + 1])
            nc.vector.tensor_add(s, s, Xh[:, m, :])
            stats = epi.tile([P, 6], F32, tag="stats")
            nc.vector.bn_stats(stats, s)
            mv = epi.tile([P, 2], F32, tag="mv")
            nc.vector.bn_aggr(mv, stats)
            sd = epi.tile([P, 1], F32, tag="sd")
            nc.scalar.activation(sd, mv[:, 1:2], AF.Sqrt, bias=eps_t[:])
            nc.vector.reciprocal(sd, sd)
            nc.vector.tensor_scalar(
                s, s, scalar1=mv[:, 0:1], scalar2=sd,
                op0=OP.subtract, op1=OP.mult,
            )
            nc.vector.tensor_mul(s, s, gb)
            nc.vector.tensor_add(Xh[:, m, :], s, bb)
            if layer == 0:
                nc.vector.tensor_scalar_mul(Xhs[:, m, :], Xh[:, m, :], dvi[:, m:m + 1])
            else:
                nc.vector.tensor_copy(Xhs[:, m, :], Xh[:, m, :])

    # ---- pooling + classifier + softmax ----
    # Xh_f.T (into QT), y = Xh_f @ Wc.T
    for m in range(VK):
        nc.sync.dma_start(
            out=QT[:, :, m * P:(m + 1) * P], in_=Xhs[:, m, :], transpose=True
        )
    for m in range(VK):
        ps = psum_sm.tile([P, NCLS], F32, tag="psy")
        for kt in range(HK):
            nc.tensor.matmul(
                ps, QT[:, kt, m * P:(m + 1) * P], WcT[:, kt, 0:NCLS],
                start=(kt == 0), stop=(kt == HK - 1),
            )
        nc.scalar.copy(ybf[:, m, :], ps)
    # pooled = H.T @ y (partial over local V)
    for m in range(EK):
        ps = psum_sm.tile([P, NCLS], F32, tag="psy")
        for k in range(VK):
            nc.tensor.matmul(
                ps, H_sb[:, k, m * P:(m + 1) * P], ybf[:, k, :],
                start=(k == 0), stop=(k == VK - 1),
            )
        nc.scalar.copy(pooled[:, m, :], ps)
    nc.sync.dma_start(out=pl_in[:], in_=pooled[:])
    nc.gpsimd.collective_compute(
        "AllReduce", OP.add, replica_groups=RG, ins=[pl_in.opt()], outs=[pl_out.opt()]
    )
    nc.sync.dma_start(out=pooled[:], in_=pl_out[:])
    # logits = pooled/cnt + bc ; softmax over the class pair
    nc.vector.tensor_mul(pooled[:], pooled[:], cnt2[:])
    nc.vector.tensor_add(pooled[:], pooled[:], bc2[:])
    nc.scalar.activation(outsb[:], pooled[:], AF.Exp)
    ssum = epi.tile([P, EK], F32, tag="ssum")
    nc.vector.tensor_add(ssum, outsb[:, :, 0], outsb[:, :, 1])
    nc.vector.reciprocal(ssum, ssum)
    for c in range(NCLS):
        nc.vector.tensor_mul(outsb[:, :, c], outsb[:, :, c], ssum)
    nc.sync.dma_start(
        out=out_d.rearrange("(et p) c -> p et c", p=P),
        in_=outsb[:],
    )
    ctx.close()


_NC_CACHE = None


def _build():
    global _NC_CACHE
    if _NC_CACHE is not None:
        return _NC_CACHE
    nc = bacc.Bacc("TRN2", target_bir_lowering=False, debug=False, num_devices=NCORES)
    with tile.TileContext(nc) as tc:
        _body(tc)
    nc.compile()
    _NC_CACHE = nc
    return nc


def kernel(**inputs):
    X = np.ascontiguousarray(np.asarray(inputs["X"], dtype=np.float32))
    H = np.ascontiguousarray(np.asarray(inputs["H_inc"], dtype=np.float32))
    full = {
        "Wp": inputs["Wp"], "W0": inputs["W0"], "W1": inputs["W1"],
        "g0": inputs["g0"], "b0": inputs["b0"],
        "g1": inputs["g1"], "b1": inputs["b1"],
        "Wc": inputs["Wc"], "bc": inputs["bc"],
    }
    full = {k: np.ascontiguousarray(np.asarray(v, dtype=np.float32))
            for k, v in full.items()}
    nc = _build()
    in_maps = []
    for c in range(NCORES):
        sl = slice(c * VL, (c + 1) * VL)
        m = {"X": X[sl], "H": H[sl]}
        m.update(full)
        in_maps.append(m)
    res = run_bass_kernel_spmd(
        nc, in_maps, core_ids=list(range(NCORES)), **_RUN_KWARGS
    )
    if _LAST_RESULT is not None:
        _LAST_RESULT.clear()
        _LAST_RESULT.append(res)
    return np.asarray(res.results[0]["out"], dtype=np.float32)


# test-harness hooks (unused during grading)
_RUN_KWARGS = {}
_LAST_RESULT = None
